# revision 25
# baseline (speedup 1.0000x reference)
"""Multi-head graph attention on 8 Trainium2 NeuronCores.

Strategy: shard destination nodes (and their incoming edges) across the 8
cores. The host pre-stages pure layout transforms of the inputs per core:
x rows for each edge's src and dst (dst-sorted, transposed, bf16), the
matching edge_attr rows, and the fp8 one-hot scatter matrices derived from
edge_index. The device then streams groups of 4x128 edges:
  - K|V and Q projected per edge on PE (x-row tiles stationary, weights
    moving); edge bias folded into an extra contraction row of the We
    matmul; V channels interleaved (dh-major) so the attention broadcast
    multiply runs in the DVE 2x mode
  - K|V evacuated to SBUF by the scalar engine; logits on DVE (mult) +
    DVE/GPSIMD (grouped reduce, alternating); exp on scalar engine
  - numerator|denominator scatter-added per 128-dst-node block via
    staged-one-hot matmul accumulation in PSUM
  - normalize, PE-transpose, project through (row-permuted) Wo
No collectives and no gathers: each core owns its output rows exclusively
and all DMA is large and sequential.
"""

import numpy as np
import ml_dtypes

D, H, ED = 128, 8, 64
DH = D // H
SCALE = DH ** -0.5
F32 = np.float32
BF16 = ml_dtypes.bfloat16
FP8 = ml_dtypes.float8_e4m3

# interleaved channel order for V / output: new index d' holds old channel
# (d' % 8) * 16 + d' // 8  (head-minor), so attn[h] broadcast over dh is a
# packed stride-1 run of 8 in the free dim.
PERM = np.array([(d % 8) * 16 + d // 8 for d in range(128)], np.int64)


class Cfg:
    def __init__(self, N=50000, E=600000, ncores=8, sb=4, grp=4):
        self.N, self.E, self.NCORES = N, E, ncores
        self.NPC = N // ncores
        self.NBLK = (self.NPC + 127) // 128
        self.NPAD = self.NBLK * 128
        self.SB = sb      # blocks per staging superblock
        self.GRP = grp    # tiles per compute group


CFG = Cfg()


def _preprocess(edge_index, cfg=CFG):
    src = np.asarray(edge_index[0], np.int64)
    dst = np.asarray(edge_index[1], np.int64)
    order = np.argsort(dst, kind="stable")
    src_s, dst_s, eid_s = src[order], dst[order], order

    core_of = dst_s // cfg.NPC
    per = [[None] * cfg.NBLK for _ in range(cfg.NCORES)]
    for c in range(cfg.NCORES):
        m = core_of == c
        sc, dc, ec = src_s[m], dst_s[m], eid_s[m]
        ld = dc - c * cfg.NPC
        blk = ld // 128
        for b in range(cfg.NBLK):
            mb = blk == b
            per[c][b] = (sc[mb], dc[mb], ld[mb] - b * 128, ec[mb])

    Tb = [max(1, max((len(per[c][b][0]) + 127) // 128
                     for c in range(cfg.NCORES)))
          for b in range(cfg.NBLK)]
    T = sum(Tb)
    gt = np.zeros(cfg.NBLK, np.int64)
    gt[1:] = np.cumsum(Tb)[:-1]

    sbs = []
    for b0 in range(0, cfg.NBLK, cfg.SB):
        b1 = min(b0 + cfg.SB, cfg.NBLK)
        sbs.append((b0, b1, int(gt[b0]), int(gt[b1 - 1]) + Tb[b1 - 1]))
    SBT = max(t1 - t0 for (_, _, t0, t1) in sbs)

    srcs = np.zeros((cfg.NCORES, T * 128), np.int64)
    dstg = np.zeros((cfg.NCORES, T * 128), np.int64)
    eids = np.full((cfg.NCORES, T * 128), -1, np.int64)
    ld_all = np.full((cfg.NCORES, T * 128), -1, np.int64)
    for c in range(cfg.NCORES):
        for b in range(cfg.NBLK):
            sc, dc, lb, ec = per[c][b]
            o = gt[b] * 128
            n = len(sc)
            srcs[c, o:o + n] = sc
            dstg[c, o:o + n] = dc
            eids[c, o:o + n] = ec
            ld_all[c, o:o + n] = lb

    return dict(Tb=Tb, T=T, gt=gt, sbs=sbs, SBT=SBT,
                srcs=srcs, dstg=dstg, eids=eids, ld_all=ld_all)


def _build_program(plan, cfg=CFG, repeat=1, parts="stage,fill,elt,scat,norm,proj",
                   rep_barrier=False, use_bq=False, dma_split_first=True,
                   red_mod=2, lag=8):
    import concourse.bacc as bacc
    import concourse.tile as tile
    import concourse.bass as bass
    import concourse.mybir as mybir

    f32, bf16, fp8 = mybir.dt.float32, mybir.dt.bfloat16, mybir.dt.float8e4
    Alu, Act = mybir.AluOpType, mybir.ActivationFunctionType
    T, Tb, gt, sbs, SBT = (plan["T"], plan["Tb"], plan["gt"], plan["sbs"],
                           plan["SBT"])
    G = cfg.GRP
    P = set(parts.split(","))

    nc = bacc.Bacc("TRN2", target_bir_lowering=False, debug=False,
                   enable_asserts=False, num_devices=cfg.NCORES)

    def din(name, shape, dt):
        return nc.dram_tensor(name, list(shape), dt, kind="ExternalInput").ap()

    xsT = din("xsT", [128, T * 128], bf16)
    xdT = din("xdT", [128, T * 128], bf16)
    ea65 = din("ea65", [65, T * 128], bf16)
    s4in = din("s4in", [128, T * 128], fp8)
    Wkv = din("Wkv", [128, 256], bf16)
    We65 = din("We65", [65, 128], bf16)
    Wq_ = din("Wq_", [128, 128], bf16)
    Wo_ = din("Wo_", [128, 128], bf16)
    bo2 = din("bo2", [1, 128], f32)
    ones_row = din("ones_row", [1, 512], f32)
    ident_in = din("ident_in", [128, 128], f32)
    if use_bq:
        ones16_in = din("ones16_in", [1, 128], bf16)
        bq4_in = din("bq4_in", [1, G * 128], bf16)
    outT = nc.dram_tensor("outT", [128, cfg.NPAD], f32,
                          kind="ExternalOutput").ap()

    def vw(a, dims, off=0):
        """View AP `a` with replaced free dims [[step, count], ...] plus an
        element offset into the free space."""
        return bass.AP(a.tensor, a.offset + off,
                       [list(a.ap[0])] + [list(d) for d in dims])

    # flat list of compute groups: (block, g0, nt, first, last)
    groups = []
    for b in range(cfg.NBLK):
        for g0 in range(0, Tb[b], G):
            nt = min(G, Tb[b] - g0)
            groups.append((b, g0, nt, g0 == 0, g0 + nt == Tb[b]))
    sb_of_block = {}
    for si, (b0, b1, t0, t1) in enumerate(sbs):
        for b in range(b0, b1):
            sb_of_block[b] = si

    with tile.TileContext(nc) as tc:
        with tc.tile_pool(name="const", bufs=1) as cpool:
            def cin(tag, shape, dt, src):
                t = cpool.tile(shape, dt, tag=tag, name=tag)
                nc.sync.dma_start(out=t[:], in_=src)
                return t

            Wkv_sb = cin("Wkv", [128, 256], bf16, Wkv[:])
            We_sb = cin("We", [65, 128], bf16, We65[:])
            Wq_sb = cin("Wq", [128, 128], bf16, Wq_[:])
            Wo_sb = cin("Wo", [128, 128], bf16, Wo_[:])
            bo_sb = cin("bo", [1, 128], f32, bo2[:])
            ones_sb = cin("ones", [1, 512], f32, ones_row[:])
            ident_sb = cin("ident", [128, 128], f32, ident_in[:])
            if use_bq:
                ones16_sb = cin("ones16", [1, 128], bf16, ones16_in[:])
                bq4_sb = cin("bq4", [1, G * 128], bf16, bq4_in[:])
            oT_all = cpool.tile([128, cfg.NPAD], bf16, tag="oT_all",
                                name="oT_all")

            for _rep in range(repeat):
                with tc.tile_pool(name="stg", bufs=2) as stg, \
                     tc.tile_pool(name="wk", bufs=2 + lag) as wk, \
                     tc.tile_pool(name="nrm", bufs=2) as nrm, \
                     tc.tile_pool(name="pkv", bufs=2, space="PSUM") as pkv, \
                     tc.tile_pool(name="pud", bufs=2, space="PSUM") as pud:

                    sb_tiles = {}

                    def load_sb(si):
                        b0, b1, t0, t1 = sbs[si]
                        xs = stg.tile([128, SBT * 128], bf16, tag="xs",
                                      name="xs")
                        xd = stg.tile([128, SBT * 128], bf16, tag="xd",
                                      name="xd")
                        ea = stg.tile([65, SBT * 128], bf16, tag="ea",
                                      name="ea")
                        s4 = stg.tile([128, SBT * 128], fp8, tag="s4",
                                      name="s4")
                        if "stage" in P:
                            if si == 0 and dma_split_first:
                                blocks = [(int(gt[b]) * 128, Tb[b] * 128,
                                           (int(gt[b]) - t0) * 128)
                                          for b in range(b0, b1)]
                            else:
                                blocks = [(t0 * 128, (t1 - t0) * 128, 0)]
                            for (s0, n, o) in blocks:
                                src = slice(s0, s0 + n)
                                dst = slice(o, o + n)
                                nc.sync.dma_start(out=xs[:, dst],
                                                  in_=xsT[:, src])
                                nc.sync.dma_start(out=xd[:, dst],
                                                  in_=xdT[:, src])
                                nc.sync.dma_start(out=ea[:, dst],
                                                  in_=ea65[:, src])
                                nc.sync.dma_start(out=s4[:, dst],
                                                  in_=s4in[:, src])
                        return xs, xd, ea, s4

                    # software-pipelined emission:
                    #   PE:   fill(i), scatter(i-2)
                    #   ACT:  kvevac(i), exp(i-1)
                    #   DVE:  vmult(i-1), mult(i)
                    #   Pool: reduce(i) on odd groups (DVE on even)
                    sb_loaded = -1
                    hist = []      # per-group state dicts, index-aligned
                    ud_cur = [None]

                    def fill(i):
                        b, g0, nt, first, last = groups[i]
                        si = sb_of_block[b]
                        nonlocal sb_loaded
                        if si != sb_loaded:
                            sb_tiles[si] = load_sb(si)
                            if sb_loaded >= 0 and sb_loaded - 1 in sb_tiles:
                                del sb_tiles[sb_loaded - 1]
                            sb_loaded = si
                        xs_t, xd_t, ea_t, s4_t = sb_tiles[si]
                        ts0 = int(gt[b] - sbs[si][2] + g0) * 128
                        kv_ps = pkv.tile([128, G, 256], f32, tag="kv",
                                         name="kv_ps")
                        q_ps = pkv.tile([128, G, 128], f32, tag="q",
                                        name="q_ps")
                        if "fill" in P:
                            for t in range(nt):
                                sl = slice(ts0 + t * 128, ts0 + (t + 1) * 128)
                                nc.tensor.matmul(out=kv_ps[:, t, :],
                                                 lhsT=xs_t[:, sl],
                                                 rhs=Wkv_sb[:],
                                                 start=True, stop=False)
                                nc.tensor.matmul(out=kv_ps[:, t, 0:128],
                                                 lhsT=ea_t[:65, sl],
                                                 rhs=We_sb[:],
                                                 start=False, stop=True,
                                                 skip_group_check=True)
                                nc.tensor.matmul(out=q_ps[:, t, :],
                                                 lhsT=xd_t[:, sl],
                                                 rhs=Wq_sb[:],
                                                 start=True,
                                                 stop=not use_bq)
                            if use_bq:
                                nc.tensor.matmul(
                                    out=vw(q_ps[:], [[1, nt * 128]]),
                                    lhsT=ones16_sb[:],
                                    rhs=bq4_sb[:, :nt * 128],
                                    start=False, stop=True,
                                    skip_group_check=True)
                        return dict(i=i, b=b, g0=g0, nt=nt, first=first,
                                    last=last, kv_ps=kv_ps, q_ps=q_ps,
                                    s4_t=s4_t, ts0=ts0)

                    def kvevac_mult(st):
                        nt = st["nt"]
                        kv_sb = wk.tile([128, G, 256], bf16, tag="kv_sb",
                                        name="kv_sb")
                        qw4 = wk.tile([128, G, 128], bf16, tag="qw4",
                                      name="qw4")
                        if "elt" in P:
                            nc.scalar.activation(
                                out=vw(kv_sb[:], [[1, nt * 256]]),
                                in_=vw(st["kv_ps"][:], [[1, nt * 256]]),
                                func=Act.Copy)
                            nc.vector.tensor_tensor(
                                out=vw(qw4[:], [[1, nt * 128]]),
                                in0=vw(st["q_ps"][:], [[1, nt * 128]]),
                                in1=vw(kv_sb[:], [[256, nt], [1, 128]]),
                                op=Alu.mult)
                        st["kv_sb"], st["qw4"] = kv_sb, qw4

                    def reduce(st):
                        nt = st["nt"]
                        l4 = wk.tile([128, 32], f32, tag="l4", name="l4")
                        qw = st["qw4"]
                        dve_full = red_mod == 1 or (red_mod == 2
                                                    and st["i"] % 2 == 0)
                        if red_mod == 5:
                            dve_full = False
                        if "elt" in P:
                            if dve_full:
                                nc.vector.reduce_sum(
                                    out=vw(l4[:], [[1, nt * 8]]),
                                    in_=vw(qw[:], [[16, nt * 8], [1, 16]]),
                                    axis=mybir.AxisListType.X)
                            elif red_mod == 3 and st["i"] % 2 == 1:
                                # hybrid: Pool halves 16 -> 8, DVE sums 8 -> 1
                                r8 = wk.tile([128, G * 64], f32, tag="r8t",
                                             name="r8t")
                                nc.gpsimd.tensor_tensor(
                                    out=vw(r8[:], [[1, nt * 64]]),
                                    in0=vw(qw[:], [[16, nt * 8], [1, 8]]),
                                    in1=vw(qw[:], [[16, nt * 8], [1, 8]],
                                           off=8),
                                    op=Alu.add)
                                nc.vector.reduce_sum(
                                    out=vw(l4[:], [[1, nt * 8]]),
                                    in_=vw(r8[:], [[8, nt * 8], [1, 8]]),
                                    axis=mybir.AxisListType.X)
                            else:
                                # full reduce on Pool: 4-step pairwise tree
                                r8 = wk.tile([128, G * 64], f32, tag="r8t",
                                             name="r8t")
                                r4 = wk.tile([128, G * 32], f32, tag="r4t",
                                             name="r4t")
                                r2 = wk.tile([128, G * 16], f32, tag="r2t",
                                             name="r2t")
                                nc.gpsimd.tensor_tensor(
                                    out=vw(r8[:], [[1, nt * 64]]),
                                    in0=vw(qw[:], [[16, nt * 8], [1, 8]]),
                                    in1=vw(qw[:], [[16, nt * 8], [1, 8]],
                                           off=8),
                                    op=Alu.add)
                                nc.gpsimd.tensor_tensor(
                                    out=vw(r4[:], [[1, nt * 32]]),
                                    in0=vw(r8[:], [[8, nt * 8], [1, 4]]),
                                    in1=vw(r8[:], [[8, nt * 8], [1, 4]],
                                           off=4),
                                    op=Alu.add)
                                nc.gpsimd.tensor_tensor(
                                    out=vw(r2[:], [[1, nt * 16]]),
                                    in0=vw(r4[:], [[4, nt * 8], [1, 2]]),
                                    in1=vw(r4[:], [[4, nt * 8], [1, 2]],
                                           off=2),
                                    op=Alu.add)
                                nc.gpsimd.tensor_tensor(
                                    out=vw(l4[:], [[1, nt * 8]]),
                                    in0=vw(r2[:], [[2, nt * 8]]),
                                    in1=vw(r2[:], [[2, nt * 8]], off=1),
                                    op=Alu.add)
                        st["l4"] = l4

                    def exp(st):
                        nt = st["nt"]
                        rhs4 = wk.tile([128, G, 136], bf16, tag="rhs4",
                                       name="rhs4")
                        if "elt" in P:
                            nc.scalar.activation(
                                out=vw(rhs4[:], [[136, nt], [1, 8]], off=128),
                                in_=vw(st["l4"][:], [[8, nt], [1, 8]]),
                                func=Act.Exp, scale=float(SCALE))
                        st["rhs4"] = rhs4

                    def vmult(st):
                        nt = st["nt"]
                        if "elt" in P:
                            nc.vector.tensor_tensor(
                                out=vw(st["rhs4"][:], [[136, nt], [1, 128]]),
                                in0=vw(st["kv_sb"][:], [[256, nt], [1, 128]],
                                       off=128),
                                in1=vw(st["rhs4"][:],
                                       [[136, nt], [0, 16], [1, 8]], off=128),
                                op=Alu.mult)

                    def scatter(st):
                        b, g0, nt = st["b"], st["g0"], st["nt"]
                        if st["first"] and "scat" in P:
                            ud_cur[0] = pud.tile([128, 400], f32, tag="ud",
                                                 name="ud")
                        ud = ud_cur[0]
                        if "scat" in P:
                            for t in range(nt):
                                sl = slice(st["ts0"] + t * 128,
                                           st["ts0"] + (t + 1) * 128)
                                nc.tensor.matmul(
                                    out=ud[:, 0:136],
                                    lhsT=st["s4_t"][:, sl],
                                    rhs=st["rhs4"][:, t, :],
                                    start=(st["first"] and t == 0),
                                    stop=(st["last"] and t == nt - 1),
                                    skip_group_check=True)
                        if st["last"] and "norm" in P:
                            d8 = nrm.tile([128, 8], f32, tag="d8", name="d8")
                            nc.vector.tensor_copy(out=d8[:],
                                                  in_=ud[:, 128:136])
                            nc.vector.tensor_scalar_max(d8[:], d8[:], 1e-30)
                            r8 = nrm.tile([128, 8], f32, tag="r8", name="r8")
                            nc.vector.reciprocal(r8[:], d8[:])
                            o_sb = nrm.tile([128, 128], f32, tag="o_sb",
                                            name="o_sb")
                            nc.vector.tensor_tensor(
                                out=vw(o_sb[:], [[1, 128]]),
                                in0=vw(ud, [[1, 128]]),
                                in1=vw(r8[:], [[0, 16], [1, 8]]),
                                op=Alu.mult)
                            nc.tensor.transpose(out=ud[:, 136:264],
                                                in_=o_sb[:],
                                                identity=ident_sb[:])
                            nc.scalar.activation(
                                out=oT_all[:, b * 128:(b + 1) * 128],
                                in_=ud[:, 136:264], func=Act.Copy)
                        if st["last"] and "proj" in P:
                            nc.tensor.matmul(
                                out=ud[:, 272:400], lhsT=Wo_sb[:],
                                rhs=oT_all[:, b * 128:(b + 1) * 128],
                                start=True, stop=False)
                            nc.tensor.matmul(
                                out=ud[:, 272:400], lhsT=bo_sb[:],
                                rhs=ones_sb[:, :128], start=False, stop=True)
                            ot = wk.tile([128, 128], f32, tag="ot",
                                         name="ot")
                            nc.scalar.activation(out=ot[:],
                                                 in_=ud[:, 272:400],
                                                 func=Act.Copy)
                            nc.sync.dma_start(
                                out=outT[:, b * 128:(b + 1) * 128],
                                in_=ot[:])

                    NG = len(groups)
                    for i in range(NG + lag + 1):
                        if i < NG:
                            st = fill(i)
                            hist.append(st)
                        if 0 <= i - lag < NG:
                            exp(hist[i - lag])
                            vmult(hist[i - lag])
                        if i < NG:
                            kvevac_mult(st)
                            reduce(st)
                        if 0 <= i - lag - 1 < NG:
                            scatter(hist[i - lag - 1])
                            hist[i - lag - 1] = None
                if rep_barrier:
                    tc.strict_bb_all_engine_barrier()

    nc.compile()
    return nc


def _make_inputs(plan, x, edge_attr, Wq, bq, Wk, bk, Wv, bv, We, be, Wo, bo,
                 cfg=CFG):
    T = plan["T"]
    x_bf = np.asarray(x, F32).astype(BF16)
    ea_bf = np.asarray(edge_attr, F32).astype(BF16)

    Wv_p = np.asarray(Wv, F32)[:, PERM]
    Wkv = np.concatenate([np.asarray(Wk, F32), Wv_p], axis=1).astype(BF16)
    We65 = np.concatenate(
        [np.asarray(We, F32),
         (np.asarray(be, F32) + np.asarray(bk, F32))[None, :]],
        axis=0).astype(BF16)
    Wo_p = np.asarray(Wo, F32)[PERM, :]
    bo2 = (np.asarray(bo, F32) +
           np.asarray(bv, F32) @ np.asarray(Wo, F32))[None, :]
    use_bq = bool(np.any(np.asarray(bq, F32)))

    common = {
        "Wkv": Wkv, "We65": We65,
        "Wq_": np.asarray(Wq, F32).astype(BF16),
        "Wo_": Wo_p.astype(BF16),
        "bo2": bo2,
        "ones_row": np.ones((1, 512), F32),
        "ident_in": np.eye(128, dtype=F32),
    }
    if use_bq:
        common["ones16_in"] = np.ones((1, 128), BF16)
        common["bq4_in"] = np.tile(np.asarray(bq, F32), cfg.GRP)[None, :] \
            .astype(BF16)

    jj = np.arange(128, dtype=np.int64)
    in_maps = []
    for c in range(cfg.NCORES):
        xsT = np.ascontiguousarray(x_bf[plan["srcs"][c]].T)
        xdT = np.ascontiguousarray(x_bf[plan["dstg"][c]].T)
        eid = plan["eids"][c]
        ea_rows = np.zeros((T * 128, 65), BF16)
        valid = eid >= 0
        ea_rows[valid, :ED] = ea_bf[eid[valid]]
        ea_rows[:, ED] = 1.0
        ea65 = np.ascontiguousarray(ea_rows.T)
        ld = plan["ld_all"][c].reshape(T, 128)
        oh = (ld[:, :, None] == jj).astype(FP8)        # [T, e, j]
        s4 = np.ascontiguousarray(
            oh.transpose(1, 0, 2).reshape(128, T * 128))
        in_maps.append(dict(common, xsT=xsT, xdT=xdT, ea65=ea65, s4in=s4))
    return in_maps


def _assemble(results, cfg=CFG):
    out = np.empty((cfg.N, D), F32)
    for c in range(cfg.NCORES):
        out[c * cfg.NPC:(c + 1) * cfg.NPC] = \
            np.asarray(results[c]["outT"])[:, :cfg.NPC].T
    return out


def kernel(x, edge_attr, Wq, bq, Wk, bk, Wv, bv, We, be, Wo, bo, edge_index):
    from concourse import bass_utils

    cfg = CFG
    plan = _preprocess(np.asarray(edge_index), cfg)
    use_bq = bool(np.any(np.asarray(bq, F32)))
    nc = _build_program(plan, cfg, use_bq=use_bq)
    in_maps = _make_inputs(plan, x, edge_attr, Wq, bq, Wk, bk, Wv, bv,
                           We, be, Wo, bo, cfg)
    res = bass_utils.run_bass_kernel_spmd(nc, in_maps,
                                          core_ids=list(range(cfg.NCORES)))
    return _assemble(res.results, cfg)
